# revision 9
# baseline (speedup 1.0000x reference)
"""Trainium2 Bass kernel for GCNN message passing.

out[b] = relu((A @ x[b]) @ W + bias),  A sparse [N, N] from 800k edges.

Sharding (8 NeuronCores): core i = (batch b = i//2, row-half h = i%2).
Each core computes output rows [h*25088, h*25088+25088) of batch b.

Device algorithm per core (all f32):
  Host pre-sorts this half's edges by destination row into 196 row-blocks of
  128 rows. Within a block, edges are split into "low" (col < 32768) and
  "high" (col >= 32768) groups so gather indices fit in int16, padded to a
  uniform L / H tiles of 128 edges (col=0/val=0 padding).
  Per row-block:
    - two dma_gather instructions (low base x[0:], high base x[32768:])
      fetch the neighbor rows x[col[e], :] into msgs [128, L+H, 128]:
      edge slot k -> partition k%128, tile k//128.
    - per edge-tile t, DVE builds the scaled one-hot scatter matrix
      S[e, r] = (iota[r] == row_local[e]) * val[e]   [128, 128]
    - PE accumulates aggT[feat, row] += msgs[:, t, :].T @ S in PSUM
      (matmul start/stop over the T tiles = segment sum).
    - PE then computes outT[o, r] = W.T @ aggT (W as stationary lhsT),
      ACT applies relu(. + bias[o]) into an SBUF staging tile,
      batched DMA writes outT [128, 25088] back to HBM.
  Host transposes/concatenates the 8 per-core outputs.
"""
import sys

import numpy as np

try:  # concourse (Bass) lives in the trn repo
    import concourse  # noqa: F401
except ImportError:  # pragma: no cover
    sys.path.insert(0, "/opt/trn_rl_repo")

B, N, E, C = 4, 50000, 800000, 128
LAST_RESULTS = None  # BassKernelResults of the most recent kernel() call
P = 128
RB = 196            # row-blocks per core (half)
RH = RB * P         # padded rows per half = 25088
SPLIT = 32768       # low/high column split for int16 gather indices
OUT_DMA_BLKS = 8    # row-blocks per output DMA


def _pack_idx(vals, n_slots):
    """Pack indices into the dma_gather int16 layout: index k at
    [k % 16, k // 16], replicated to 128 partitions. -> [128, n_slots//16]"""
    buf = np.zeros(n_slots, np.int16)
    buf[:len(vals)] = vals
    tile16 = buf.reshape(n_slots // 16, 16).T       # [16, n_slots/16]
    return np.tile(tile16, (8, 1))                  # [128, n_slots/16]


def _preprocess(edge_row, edge_col, edge_vals):
    """Build per-half gather-index / row-local / val tables.

    Returns (lowidx [2, RB, 128, 8L], highidx [2, RB, 128, 8H],
             rl [2, RB, P, T], val [2, RB, P, T], L, H) with T = L + H.
    Edge slot k of a block: partition k%128, tile k//128; slots < L*128 are
    low-group (col < SPLIT), the rest high-group (col - SPLIT).
    """
    per_half = []
    maxlow = maxhigh = 0
    for h in range(2):
        lo, hi = h * RH, min((h + 1) * RH, N)
        m = (edge_row >= lo) & (edge_row < hi)
        r, c, v = edge_row[m] - lo, edge_col[m], edge_vals[m]
        is_high = c >= SPLIT
        # sort by (block, is_high) keeping row order within groups
        order = np.lexsort((is_high, r // P))
        r, c, v, is_high = r[order], c[order], v[order], is_high[order]
        blocks = []
        for blk in range(RB):
            sel = slice(*np.searchsorted(r // P, [blk, blk + 1]))
            rb, cb, vb, hb = r[sel], c[sel], v[sel], is_high[sel]
            nlow = int((~hb).sum())
            blocks.append((rb, cb, vb, nlow))
            maxlow = max(maxlow, nlow)
            maxhigh = max(maxhigh, len(rb) - nlow)
        per_half.append(blocks)
    L = (maxlow + P - 1) // P
    H = (maxhigh + P - 1) // P
    T = L + H
    lowidx = np.zeros((2, RB, P, 8 * L), np.int16)
    highidx = np.zeros((2, RB, P, 8 * H), np.int16)
    rl = np.zeros((2, RB, P, T), np.float32)
    val = np.zeros((2, RB, P, T), np.float32)
    for h in range(2):
        for blk in range(RB):
            rb, cb, vb, nlow = per_half[h][blk]
            lowidx[h, blk] = _pack_idx(cb[:nlow], L * P)
            highidx[h, blk] = _pack_idx(cb[nlow:] - SPLIT, H * P)
            rr = np.zeros(T * P, np.float32)
            vv = np.zeros(T * P, np.float32)
            rr[:nlow] = (rb[:nlow] - blk * P).astype(np.float32)
            vv[:nlow] = vb[:nlow]
            nh = len(rb) - nlow
            rr[L * P:L * P + nh] = (rb[nlow:] - blk * P).astype(np.float32)
            vv[L * P:L * P + nh] = vb[nlow:]
            # slot k = t*P + p  ->  [p, t]
            rl[h, blk] = rr.reshape(T, P).T
            val[h, blk] = vv.reshape(T, P).T
    return lowidx, highidx, rl, val, L, H


def _build_program(L, H, n_blocks=RB, n_rows=N):
    import concourse.bacc as bacc
    import concourse.tile as tile
    from concourse import mybir
    from concourse._compat import get_trn_type

    T = L + H
    f32 = mybir.dt.float32
    i16 = mybir.dt.int16
    nc = bacc.Bacc(get_trn_type() or "TRN2", target_bir_lowering=False)

    x_d = nc.dram_tensor("x", [n_rows, C], f32, kind="ExternalInput")
    lowidx_d = nc.dram_tensor("lowidx", [P, n_blocks * 8 * L], i16,
                              kind="ExternalInput")
    highidx_d = nc.dram_tensor("highidx", [P, n_blocks * 8 * H], i16,
                               kind="ExternalInput")
    rl_d = nc.dram_tensor("rl", [P, n_blocks * T], f32, kind="ExternalInput")
    val_d = nc.dram_tensor("val", [P, n_blocks * T], f32, kind="ExternalInput")
    wt_d = nc.dram_tensor("wt", [C, C], f32, kind="ExternalInput")
    bias_d = nc.dram_tensor("bias", [C, 1], f32, kind="ExternalInput")
    iota_d = nc.dram_tensor("iota", [P, P], f32, kind="ExternalInput")
    out_d = nc.dram_tensor("outT", [C, n_blocks * P], f32,
                           kind="ExternalOutput")

    with tile.TileContext(nc) as tc:
        with (
            tc.tile_pool(name="const", bufs=1) as const_pool,
            tc.tile_pool(name="meta", bufs=1) as meta_pool,
            tc.tile_pool(name="msgs", bufs=3) as msgs_pool,
            tc.tile_pool(name="smat", bufs=4) as s_pool,
            tc.tile_pool(name="aggsb", bufs=2) as agg_pool,
            tc.tile_pool(name="ostage", bufs=2) as ostage_pool,
            tc.tile_pool(name="psum_agg", bufs=2, space="PSUM") as psA,
            tc.tile_pool(name="psum_out", bufs=2, space="PSUM") as psO,
        ):
            wt_sb = const_pool.tile([C, C], f32)
            bias_sb = const_pool.tile([C, 1], f32)
            iota_sb = const_pool.tile([P, P], f32)
            nc.sync.dma_start(out=wt_sb[:], in_=wt_d[:])
            nc.sync.dma_start(out=bias_sb[:], in_=bias_d[:])
            nc.sync.dma_start(out=iota_sb[:], in_=iota_d[:])

            lowidx_sb = meta_pool.tile([P, n_blocks * 8 * L], i16)
            highidx_sb = meta_pool.tile([P, n_blocks * 8 * H], i16)
            rl_sb = meta_pool.tile([P, n_blocks * T], f32)
            val_sb = meta_pool.tile([P, n_blocks * T], f32)
            nc.sync.dma_start(out=lowidx_sb[:], in_=lowidx_d[:])
            nc.sync.dma_start(out=highidx_sb[:], in_=highidx_d[:])
            nc.sync.dma_start(out=rl_sb[:], in_=rl_d[:])
            nc.sync.dma_start(out=val_sb[:], in_=val_d[:])

            ostage = None
            for blk in range(n_blocks):
                msgs = msgs_pool.tile([P, T, C], f32)
                nc.gpsimd.dma_gather(
                    out_ap=msgs[:, :L, :],
                    in_ap=x_d[:SPLIT, :],
                    idxs_ap=lowidx_sb[:, blk * 8 * L:(blk + 1) * 8 * L],
                    num_idxs=L * P,
                    num_idxs_reg=L * P,
                    elem_size=C,
                    single_packet=False,
                )
                nc.gpsimd.dma_gather(
                    out_ap=msgs[:, L:, :],
                    in_ap=x_d[SPLIT:, :],
                    idxs_ap=highidx_sb[:, blk * 8 * H:(blk + 1) * 8 * H],
                    num_idxs=H * P,
                    num_idxs_reg=H * P,
                    elem_size=C,
                    single_packet=False,
                )
                aggT_ps = psA.tile([C, P], f32)
                for t in range(T):
                    s_t = s_pool.tile([P, P], f32)
                    j = blk * T + t
                    nc.vector.tensor_scalar(
                        out=s_t[:], in0=iota_sb[:],
                        scalar1=rl_sb[:, j:j + 1],
                        scalar2=val_sb[:, j:j + 1],
                        op0=mybir.AluOpType.is_equal,
                        op1=mybir.AluOpType.mult,
                    )
                    nc.tensor.matmul(
                        out=aggT_ps[:], lhsT=msgs[:, t, :], rhs=s_t[:],
                        start=(t == 0), stop=(t == T - 1),
                    )
                aggT_sb = agg_pool.tile([C, P], f32)
                nc.vector.tensor_copy(out=aggT_sb[:], in_=aggT_ps[:])
                outT_ps = psO.tile([C, P], f32)
                nc.tensor.matmul(out=outT_ps[:], lhsT=wt_sb[:],
                                 rhs=aggT_sb[:], start=True, stop=True)
                if blk % OUT_DMA_BLKS == 0:
                    ostage = ostage_pool.tile([C, OUT_DMA_BLKS * P], f32)
                o_off = (blk % OUT_DMA_BLKS) * P
                nc.scalar.activation(
                    out=ostage[:, o_off:o_off + P], in_=outT_ps[:],
                    func=mybir.ActivationFunctionType.Relu,
                    bias=bias_sb[:, :1], scale=1.0,
                )
                if blk % OUT_DMA_BLKS == OUT_DMA_BLKS - 1 or blk == n_blocks - 1:
                    lo_blk = (blk // OUT_DMA_BLKS) * OUT_DMA_BLKS
                    width = (blk - lo_blk + 1) * P
                    nc.sync.dma_start(
                        out=out_d[:, lo_blk * P: lo_blk * P + width],
                        in_=ostage[:, :width],
                    )
    return nc


def kernel(x, edge_row, edge_col, edge_vals, W, b):
    from concourse.bass_utils import run_bass_kernel_spmd

    x = np.asarray(x, np.float32)
    edge_row = np.asarray(edge_row, np.int32)
    edge_col = np.asarray(edge_col, np.int32)
    edge_vals = np.asarray(edge_vals, np.float32)
    W = np.asarray(W, np.float32)
    b = np.asarray(b, np.float32)

    lowidx, highidx, rl, val, L, H = _preprocess(edge_row, edge_col, edge_vals)
    T = L + H
    nc = _build_program(L, H)
    nc.compile()

    iota = np.tile(np.arange(P, dtype=np.float32), (P, 1))
    in_maps = []
    for core in range(8):
        bb, h = core // 2, core % 2
        in_maps.append({
            "x": np.ascontiguousarray(x[bb]),
            "lowidx": np.ascontiguousarray(
                lowidx[h].transpose(1, 0, 2).reshape(P, RB * 8 * L)),
            "highidx": np.ascontiguousarray(
                highidx[h].transpose(1, 0, 2).reshape(P, RB * 8 * H)),
            "rl": np.ascontiguousarray(
                rl[h].transpose(1, 0, 2).reshape(P, RB * T)),
            "val": np.ascontiguousarray(
                val[h].transpose(1, 0, 2).reshape(P, RB * T)),
            "wt": W,
            "bias": np.ascontiguousarray(b[:, None]),
            "iota": iota,
        })

    res = run_bass_kernel_spmd(nc, in_maps, list(range(8)))
    global LAST_RESULTS
    LAST_RESULTS = res

    out = np.empty((B, N, C), np.float32)
    for core in range(8):
        bb, h = core // 2, core % 2
        lo, hi = h * RH, min((h + 1) * RH, N)
        out[bb, lo:hi] = res.results[core]["outT"].T[:hi - lo]
    return out


# revision 13
# speedup vs baseline: 4.0062x; 4.0062x over previous
"""Trainium2 Bass kernel for GCNN message passing.

out[b] = relu((A @ x[b]) @ W + bias),  A sparse [N, N] from 800k edges.

Sharding (8 NeuronCores): core h owns output rows [h*6272, (h+1)*6272) for
ALL 4 batches. Host interleaves x into xcat[n] = x[:, n, :] (bf16,
[N, 4*128]) so ONE gather descriptor fetches a neighbor's features for all
4 batches at once (4x fewer descriptors — Q7 descriptor generation is the
bottleneck resource).

Device algorithm per core:
  Host pre-sorts the core's ~100k edges by destination row into 49
  row-blocks of 128 rows; within a block edges are split into "low"
  (col < 32768) / "high" groups so gather indices fit in int16, padded to
  uniform L / H tiles of 128 edges (col=0/val=0 padding).
  Per row-block:
    - two dma_gather ops (bases xcat[0:], xcat[32768:]) fetch
      msgs [128(edge), T, 512] bf16; edge slot k -> partition k%128,
      tile k//128.
    - per edge-tile t, DVE builds S[e, r] = (iota[r]==row_local[e])*val[e]
      in bf16 once, shared by all batches.
    - PE accumulates aggT_b[c, r] += msgs[:, t, b*128:+128].T @ S into one
      PSUM bank [128, 4*128] (segment sum via matmul accumulation).
    - aggT -> SBUF bf16, PE applies W (outT_b = W.T @ aggT_b) into a second
      PSUM bank, ACT applies relu(.+bias) for all 4 batches in one op,
      batched DMA writes outT [4, 128, 6272] f32.
  Host transposes/concatenates the 8 per-core outputs.
"""
import sys

import numpy as np

try:  # concourse (Bass) lives in the trn repo
    import concourse  # noqa: F401
except ImportError:  # pragma: no cover
    sys.path.insert(0, "/opt/trn_rl_repo")

import ml_dtypes

B, N, E, C = 4, 50000, 800000, 128
LAST_RESULTS = None  # BassKernelResults of the most recent kernel() call
P = 128
RB = 49             # row-blocks per core (eighth)
RH = RB * P         # rows per core = 6272
NCORES = 8
SPLIT = 32768       # low/high column split for int16 gather indices
OUT_DMA_BLKS = 8    # row-blocks per output DMA


def _pack_idx(vals, n_slots):
    """dma_gather int16 index layout: index k at [k % 16, k // 16],
    replicated to 128 partitions. -> [128, n_slots // 16]"""
    buf = np.zeros(n_slots, np.int16)
    buf[:len(vals)] = vals
    tile16 = buf.reshape(n_slots // 16, 16).T
    return np.tile(tile16, (8, 1))


def _preprocess(edge_row, edge_col, edge_vals):
    """Per-core gather-index / row-local / val tables.

    Returns (lowidx [8, RB, 128, 8L], highidx [8, RB, 128, 8H],
             rl [8, RB, P, T], val [8, RB, P, T], L, H), T = L + H.
    Edge slot k of a block: partition k%128, tile k//128; slots < L*128
    low-group (col), the rest high-group (col - SPLIT).
    """
    per_core = []
    maxlow = maxhigh = 0
    for h in range(NCORES):
        lo, hi = h * RH, min((h + 1) * RH, N)
        m = (edge_row >= lo) & (edge_row < hi)
        r, c, v = edge_row[m] - lo, edge_col[m], edge_vals[m]
        is_high = c >= SPLIT
        order = np.lexsort((is_high, r // P))
        r, c, v, is_high = r[order], c[order], v[order], is_high[order]
        blocks = []
        for blk in range(RB):
            sel = slice(*np.searchsorted(r // P, [blk, blk + 1]))
            rb, cb, vb, hb = r[sel], c[sel], v[sel], is_high[sel]
            nlow = int((~hb).sum())
            blocks.append((rb, cb, vb, nlow))
            maxlow = max(maxlow, nlow)
            maxhigh = max(maxhigh, len(rb) - nlow)
        per_core.append(blocks)
    L = (maxlow + P - 1) // P
    H = (maxhigh + P - 1) // P
    T = L + H
    lowidx = np.zeros((NCORES, RB, P, 8 * L), np.int16)
    highidx = np.zeros((NCORES, RB, P, 8 * H), np.int16)
    rl = np.zeros((NCORES, RB, P, T), np.float32)
    val = np.zeros((NCORES, RB, P, T), np.float32)
    for h in range(NCORES):
        for blk in range(RB):
            rb, cb, vb, nlow = per_core[h][blk]
            lowidx[h, blk] = _pack_idx(cb[:nlow], L * P)
            highidx[h, blk] = _pack_idx(cb[nlow:] - SPLIT, H * P)
            rr = np.zeros(T * P, np.float32)
            vv = np.zeros(T * P, np.float32)
            rr[:nlow] = (rb[:nlow] - blk * P).astype(np.float32)
            vv[:nlow] = vb[:nlow]
            nh = len(rb) - nlow
            rr[L * P:L * P + nh] = (rb[nlow:] - blk * P).astype(np.float32)
            vv[L * P:L * P + nh] = vb[nlow:]
            rl[h, blk] = rr.reshape(T, P).T   # slot k=t*P+p -> [p, t]
            val[h, blk] = vv.reshape(T, P).T
    return lowidx, highidx, rl, val, L, H


def _build_program(L, H, n_blocks=RB, n_rows=N):
    import concourse.bacc as bacc
    import concourse.tile as tile
    from concourse import mybir
    from concourse._compat import get_trn_type

    T = L + H
    BC = B * C                       # 512 feature cols in xcat
    f32 = mybir.dt.float32
    bf16 = mybir.dt.bfloat16
    i16 = mybir.dt.int16
    nc = bacc.Bacc(get_trn_type() or "TRN2", target_bir_lowering=False)

    x_d = nc.dram_tensor("xcat", [n_rows, BC], bf16, kind="ExternalInput")
    lowidx_d = nc.dram_tensor("lowidx", [P, n_blocks * 8 * L], i16,
                              kind="ExternalInput")
    highidx_d = nc.dram_tensor("highidx", [P, n_blocks * 8 * H], i16,
                               kind="ExternalInput")
    rl_d = nc.dram_tensor("rl", [P, n_blocks * T], f32, kind="ExternalInput")
    val_d = nc.dram_tensor("val", [P, n_blocks * T], f32, kind="ExternalInput")
    wt_d = nc.dram_tensor("wt", [C, C], bf16, kind="ExternalInput")
    bias_d = nc.dram_tensor("bias", [C, 1], f32, kind="ExternalInput")
    iota_d = nc.dram_tensor("iota", [P, P], f32, kind="ExternalInput")
    out_d = nc.dram_tensor("outT", [B, C, n_blocks * P], f32,
                           kind="ExternalOutput")

    with tile.TileContext(nc) as tc:
        with (
            tc.tile_pool(name="const", bufs=1) as const_pool,
            tc.tile_pool(name="meta", bufs=1) as meta_pool,
            tc.tile_pool(name="msgs", bufs=3) as msgs_pool,
            tc.tile_pool(name="smat", bufs=24) as s_pool,
            tc.tile_pool(name="aggsb", bufs=2) as agg_pool,
            tc.tile_pool(name="ostage", bufs=2) as ostage_pool,
            tc.tile_pool(name="psum_agg", bufs=2, space="PSUM") as psA,
            tc.tile_pool(name="psum_out", bufs=2, space="PSUM") as psO,
        ):
            wt_sb = const_pool.tile([C, C], bf16)
            bias_sb = const_pool.tile([C, 1], f32)
            iota_sb = const_pool.tile([P, P], f32)
            nc.sync.dma_start(out=wt_sb[:], in_=wt_d[:])
            nc.sync.dma_start(out=bias_sb[:], in_=bias_d[:])
            nc.sync.dma_start(out=iota_sb[:], in_=iota_d[:])

            lowidx_sb = meta_pool.tile([P, n_blocks * 8 * L], i16)
            highidx_sb = meta_pool.tile([P, n_blocks * 8 * H], i16)
            rl_sb = meta_pool.tile([P, n_blocks * T], f32)
            val_sb = meta_pool.tile([P, n_blocks * T], f32)
            nc.sync.dma_start(out=lowidx_sb[:], in_=lowidx_d[:])
            nc.sync.dma_start(out=highidx_sb[:], in_=highidx_d[:])
            nc.sync.dma_start(out=rl_sb[:], in_=rl_d[:])
            nc.sync.dma_start(out=val_sb[:], in_=val_d[:])

            ostage = None
            for blk in range(n_blocks):
                msgs = msgs_pool.tile([P, T, BC], bf16)
                nc.gpsimd.dma_gather(
                    out_ap=msgs[:, :L, :],
                    in_ap=x_d[:SPLIT, :],
                    idxs_ap=lowidx_sb[:, blk * 8 * L:(blk + 1) * 8 * L],
                    num_idxs=L * P,
                    num_idxs_reg=L * P,
                    elem_size=BC,
                    single_packet=False,
                )
                nc.gpsimd.dma_gather(
                    out_ap=msgs[:, L:, :],
                    in_ap=x_d[SPLIT:, :],
                    idxs_ap=highidx_sb[:, blk * 8 * H:(blk + 1) * 8 * H],
                    num_idxs=H * P,
                    num_idxs_reg=H * P,
                    elem_size=BC,
                    single_packet=False,
                )
                aggT_ps = psA.tile([C, B * P], f32)
                s_tiles = []
                for t in range(T):
                    s_t = s_pool.tile([P, P], bf16)
                    j = blk * T + t
                    nc.vector.tensor_scalar(
                        out=s_t[:], in0=iota_sb[:],
                        scalar1=rl_sb[:, j:j + 1],
                        scalar2=val_sb[:, j:j + 1],
                        op0=mybir.AluOpType.is_equal,
                        op1=mybir.AluOpType.mult,
                    )
                    s_tiles.append(s_t)
                for bb in range(B):
                    for t in range(T):
                        nc.tensor.matmul(
                            out=aggT_ps[:, bb * P:(bb + 1) * P],
                            lhsT=msgs[:, t, bb * C:(bb + 1) * C],
                            rhs=s_tiles[t][:],
                            start=(t == 0), stop=(t == T - 1),
                        )
                aggT_sb = agg_pool.tile([C, B * P], bf16)
                nc.vector.tensor_copy(out=aggT_sb[:], in_=aggT_ps[:])
                outT_ps = psO.tile([C, B * P], f32)
                for bb in range(B):
                    nc.tensor.matmul(
                        out=outT_ps[:, bb * P:(bb + 1) * P],
                        lhsT=wt_sb[:],
                        rhs=aggT_sb[:, bb * P:(bb + 1) * P],
                        start=True, stop=True)
                if blk % OUT_DMA_BLKS == 0:
                    ostage = ostage_pool.tile([C, B, OUT_DMA_BLKS * P], f32)
                o_off = (blk % OUT_DMA_BLKS) * P
                for bb in range(B):
                    nc.scalar.activation(
                        out=ostage[:, bb, o_off:o_off + P],
                        in_=outT_ps[:, bb * P:(bb + 1) * P],
                        func=mybir.ActivationFunctionType.Relu,
                        bias=bias_sb[:, :1], scale=1.0,
                    )
                if blk % OUT_DMA_BLKS == OUT_DMA_BLKS - 1 or blk == n_blocks - 1:
                    lo_blk = (blk // OUT_DMA_BLKS) * OUT_DMA_BLKS
                    width = (blk - lo_blk + 1) * P
                    for bb in range(B):
                        nc.sync.dma_start(
                            out=out_d[bb, :, lo_blk * P: lo_blk * P + width],
                            in_=ostage[:, bb, :width],
                        )
    return nc


def kernel(x, edge_row, edge_col, edge_vals, W, b):
    from concourse.bass_utils import run_bass_kernel_spmd

    x = np.asarray(x, np.float32)
    edge_row = np.asarray(edge_row, np.int32)
    edge_col = np.asarray(edge_col, np.int32)
    edge_vals = np.asarray(edge_vals, np.float32)
    W = np.asarray(W, np.float32)
    b = np.asarray(b, np.float32)

    lowidx, highidx, rl, val, L, H = _preprocess(edge_row, edge_col, edge_vals)
    T = L + H
    nc = _build_program(L, H)
    nc.compile()

    # xcat[n] = x[:, n, :] flattened -> [N, 4*128] bf16
    xcat = np.ascontiguousarray(
        x.transpose(1, 0, 2).reshape(N, B * C)).astype(ml_dtypes.bfloat16)
    wt = W.astype(ml_dtypes.bfloat16)
    iota = np.tile(np.arange(P, dtype=np.float32), (P, 1))
    in_maps = []
    for h in range(NCORES):
        in_maps.append({
            "xcat": xcat,
            "lowidx": np.ascontiguousarray(
                lowidx[h].transpose(1, 0, 2).reshape(P, RB * 8 * L)),
            "highidx": np.ascontiguousarray(
                highidx[h].transpose(1, 0, 2).reshape(P, RB * 8 * H)),
            "rl": np.ascontiguousarray(
                rl[h].transpose(1, 0, 2).reshape(P, RB * T)),
            "val": np.ascontiguousarray(
                val[h].transpose(1, 0, 2).reshape(P, RB * T)),
            "wt": wt,
            "bias": np.ascontiguousarray(b[:, None]),
            "iota": iota,
        })

    res = run_bass_kernel_spmd(nc, in_maps, list(range(NCORES)))
    global LAST_RESULTS
    LAST_RESULTS = res

    out = np.empty((B, N, C), np.float32)
    for h in range(NCORES):
        lo, hi = h * RH, min((h + 1) * RH, N)
        o = res.results[h]["outT"]              # [B, C, RH]
        for bb in range(B):
            out[bb, lo:hi] = o[bb].T[:hi - lo]
    return out


# revision 23
# speedup vs baseline: 4.0863x; 1.0200x over previous
"""Trainium2 Bass kernel for GCNN message passing.

out[b] = relu((A @ x[b]) @ W + bias),  A sparse [N, N] from 800k edges.

Sharding (8 NeuronCores): core h owns output rows [h*6272, (h+1)*6272) for
ALL 4 batches. Host interleaves x into xcat[n] = x[:, n, :] (bf16,
[N, 4*128]) so ONE gather descriptor fetches a neighbor's features for all
4 batches at once (Q7 descriptor generation is the bottleneck resource, at
~8ns per gather index).

Device algorithm per core:
  Host pre-sorts the core's ~100k edges by destination row into 49
  row-blocks of 128 rows; within a block edges are split into "low"
  (col < 32768) / "high" groups so gather indices fit in int16, padded to
  uniform L / H tiles of 128 edges.  Valid indices are packed first with
  trailing -1 padding; a per-block count table + gpsimd reg_load trims the
  runtime descriptor count (num_idxs_reg).
  The scaled one-hot scatter matrices S[e, r] = (iota[r]==rl[e])*val[e]
  are PREBUILT ON HOST in bf16 and streamed in (DMA has headroom; DVE
  does not).
  Per row-block:
    - two dma_gather ops (bases xcat[0:], xcat[32768:]) fetch
      msgs [128(edge), T, 512] bf16; edge slot k -> partition k%128,
      tile k//128.
    - PE accumulates aggT_b[c, r] += msgs[:, t, b*128:+128].T @ S_t into
      one PSUM bank [128, 4*128] (segment sum via matmul accumulation).
    - aggT -> SBUF bf16, PE applies W (outT_b = W.T @ aggT_b) into a
      second PSUM bank, ACT applies relu(.+bias), batched DMA writes
      outT [4, 128, 6272] f32.
  Host transposes/concatenates the 8 per-core outputs.
"""
import sys

import numpy as np

try:  # concourse (Bass) lives in the trn repo
    import concourse  # noqa: F401
except ImportError:  # pragma: no cover
    sys.path.insert(0, "/opt/trn_rl_repo")

import ml_dtypes

B, N, E, C = 4, 50000, 800000, 128
LAST_RESULTS = None  # BassKernelResults of the most recent kernel() call
P = 128
RB = 49             # row-blocks per core (eighth)
RH = RB * P         # rows per core = 6272
NCORES = 8
SPLIT = 32768       # low/high column split for int16 gather indices
OUT_DMA_BLKS = 8    # row-blocks per output DMA
DYNAMIC_COUNTS = True


def _pack_idx(vals, n_slots):
    """dma_gather int16 index layout: index k at [k % 16, k // 16],
    replicated to 128 partitions. Valid indices first, 0-padded to the next
    multiple of 128 (= the runtime count), -1 beyond. -> [128, n_slots//16]"""
    n_cover = max((len(vals) + P - 1) // P, 1) * P
    buf = np.full(n_slots, -1, np.int16)
    buf[:n_cover] = 0
    buf[:len(vals)] = vals
    tile16 = buf.reshape(n_slots // 16, 16).T
    return np.tile(tile16, (8, 1))


def _preprocess(edge_row, edge_col, edge_vals):
    """Per-core gather-index tables, host-built S matrices, and counts.

    Returns (lowidx [8, RB, 128, 8L], highidx [8, RB, 128, 8H],
             smat [8, 128, RB*T*128] bf16, counts [8, 1, RB*2] int32, L, H).
    Edge slot k of a block: partition k%128, tile k//128; slots < L*128
    low-group (col), the rest high-group (col - SPLIT).
    S tile t of block blk lives at smat[:, (blk*T+t)*128:(blk*T+t+1)*128].
    """
    per_core = []
    maxlow = maxhigh = 0
    for h in range(NCORES):
        lo, hi = h * RH, min((h + 1) * RH, N)
        m = (edge_row >= lo) & (edge_row < hi)
        r, c, v = edge_row[m] - lo, edge_col[m], edge_vals[m]
        is_high = c >= SPLIT
        order = np.lexsort((is_high, r // P))
        r, c, v, is_high = r[order], c[order], v[order], is_high[order]
        blocks = []
        for blk in range(RB):
            sel = slice(*np.searchsorted(r // P, [blk, blk + 1]))
            rb, cb, vb, hb = r[sel], c[sel], v[sel], is_high[sel]
            nlow = int((~hb).sum())
            blocks.append((rb, cb, vb, nlow))
            maxlow = max(maxlow, nlow)
            maxhigh = max(maxhigh, len(rb) - nlow)
        per_core.append(blocks)
    L = (maxlow + P - 1) // P
    H = (maxhigh + P - 1) // P
    T = L + H
    lowidx = np.zeros((NCORES, RB, P, 8 * L), np.int16)
    highidx = np.zeros((NCORES, RB, P, 8 * H), np.int16)
    counts = np.zeros((NCORES, 1, RB * 2), np.int32)
    smat = np.zeros((NCORES, P, RB * T * P), ml_dtypes.bfloat16)
    iota = np.arange(P, dtype=np.float32)
    for h in range(NCORES):
        for blk in range(RB):
            rb, cb, vb, nlow = per_core[h][blk]
            nh = len(rb) - nlow
            lowidx[h, blk] = _pack_idx(cb[:nlow], L * P)
            highidx[h, blk] = _pack_idx(cb[nlow:] - SPLIT, H * P)
            counts[h, 0, 2 * blk] = max((nlow + P - 1) // P, 1) * P
            counts[h, 0, 2 * blk + 1] = max((nh + P - 1) // P, 1) * P
            rr = np.zeros(T * P, np.float32)
            vv = np.zeros(T * P, np.float32)
            rr[:nlow] = (rb[:nlow] - blk * P).astype(np.float32)
            vv[:nlow] = vb[:nlow]
            rr[L * P:L * P + nh] = (rb[nlow:] - blk * P).astype(np.float32)
            vv[L * P:L * P + nh] = vb[nlow:]
            # S[e, r] for slot e=t*P+p -> smat[p, (blk*T+t)*P + r]
            s_f32 = (iota[None, :] == rr[:, None]) * vv[:, None]  # [T*P, P]
            smat[h, :, blk * T * P:(blk + 1) * T * P] = (
                s_f32.reshape(T, P, P).transpose(1, 0, 2).reshape(P, T * P)
                .astype(ml_dtypes.bfloat16))
    lmin = int(counts[:, 0, 0::2].min()) // P
    hmin = int(counts[:, 0, 1::2].min()) // P
    return lowidx, highidx, smat, counts, L, H, lmin, hmin


def _build_program(L, H, lmin=0, hmin=0, n_blocks=RB, n_rows=N):
    from contextlib import ExitStack

    import concourse.bacc as bacc
    import concourse.tile as tile
    from concourse import mybir
    from concourse._compat import get_trn_type

    T = L + H
    BC = B * C                       # 512 feature cols in xcat
    f32 = mybir.dt.float32
    bf16 = mybir.dt.bfloat16
    i16 = mybir.dt.int16
    nc = bacc.Bacc(get_trn_type() or "TRN2", target_bir_lowering=False)

    x_d = nc.dram_tensor("xcat", [n_rows, BC], bf16, kind="ExternalInput")
    lowidx_d = nc.dram_tensor("lowidx", [P, n_blocks * 8 * L], i16,
                              kind="ExternalInput")
    highidx_d = nc.dram_tensor("highidx", [P, n_blocks * 8 * H], i16,
                               kind="ExternalInput")
    smat_d = nc.dram_tensor("smat", [P, n_blocks * T * P], bf16,
                            kind="ExternalInput")
    counts_d = nc.dram_tensor("counts", [1, n_blocks * 2], mybir.dt.int32,
                              kind="ExternalInput")
    wt_d = nc.dram_tensor("wt", [C, C], bf16, kind="ExternalInput")
    bias_d = nc.dram_tensor("bias", [C, 1], f32, kind="ExternalInput")
    out_d = nc.dram_tensor("outT", [B, C, n_blocks * P], f32,
                           kind="ExternalOutput")

    with ExitStack() as stack, tile.TileContext(nc) as tc:
        with (
            tc.tile_pool(name="const", bufs=1) as const_pool,
            tc.tile_pool(name="meta", bufs=1) as meta_pool,
            tc.tile_pool(name="msgs", bufs=3) as msgs_pool,
            tc.tile_pool(name="smat", bufs=3) as s_pool,
            tc.tile_pool(name="aggsb", bufs=2) as agg_pool,
            tc.tile_pool(name="ostage", bufs=2) as ostage_pool,
            tc.tile_pool(name="psum_agg", bufs=2, space="PSUM") as psA,
            tc.tile_pool(name="psum_out", bufs=2, space="PSUM") as psO,
        ):
            wt_sb = const_pool.tile([C, C], bf16)
            bias_sb = const_pool.tile([C, 1], f32)
            nc.sync.dma_start(out=wt_sb[:], in_=wt_d[:])
            nc.sync.dma_start(out=bias_sb[:], in_=bias_d[:])

            lowidx_sb = meta_pool.tile([P, n_blocks * 8 * L], i16)
            highidx_sb = meta_pool.tile([P, n_blocks * 8 * H], i16)
            counts_sb = meta_pool.tile([1, n_blocks * 2], mybir.dt.int32)
            nc.sync.dma_start(out=lowidx_sb[:], in_=lowidx_d[:])
            nc.sync.dma_start(out=highidx_sb[:], in_=highidx_d[:])
            nc.sync.dma_start(out=counts_sb[:], in_=counts_d[:])

            regs = []
            if DYNAMIC_COUNTS:
                for i in range(4):
                    regs.append(stack.enter_context(
                        nc.gpsimd.register(f"cnt{i}")))

            ostage = None
            for blk in range(n_blocks):
                msgs = msgs_pool.tile([P, T, BC], bf16)
                # zero the tiles the runtime count may skip (keeps skipped
                # slots finite; their S columns are zero)
                if lmin < L:
                    nc.vector.memset(msgs[:, lmin:L, :], 0)
                if hmin < H:
                    nc.vector.memset(msgs[:, L + hmin:, :], 0)
                if DYNAMIC_COUNTS:
                    rlo = regs[(2 * blk) % 4]
                    rhi = regs[(2 * blk + 1) % 4]
                    nc.gpsimd.reg_load(rlo, counts_sb[:1, 2 * blk:2 * blk + 1])
                    nc.gpsimd.reg_load(
                        rhi, counts_sb[:1, 2 * blk + 1:2 * blk + 2])
                    nlow_reg, nhigh_reg = rlo, rhi
                else:
                    nlow_reg, nhigh_reg = L * P, H * P
                nc.gpsimd.dma_gather(
                    out_ap=msgs[:, :L, :],
                    in_ap=x_d[:SPLIT, :],
                    idxs_ap=lowidx_sb[:, blk * 8 * L:(blk + 1) * 8 * L],
                    num_idxs=L * P,
                    num_idxs_reg=nlow_reg,
                    elem_size=BC,
                    single_packet=False,
                )
                nc.gpsimd.dma_gather(
                    out_ap=msgs[:, L:, :],
                    in_ap=x_d[SPLIT:, :],
                    idxs_ap=highidx_sb[:, blk * 8 * H:(blk + 1) * 8 * H],
                    num_idxs=H * P,
                    num_idxs_reg=nhigh_reg,
                    elem_size=BC,
                    single_packet=False,
                )
                s_blk = s_pool.tile([P, T * P], bf16)
                nc.sync.dma_start(
                    out=s_blk[:],
                    in_=smat_d[:, blk * T * P:(blk + 1) * T * P])
                aggT_ps = psA.tile([C, B * P], f32)
                for bb in range(B):
                    for t in range(T):
                        nc.tensor.matmul(
                            out=aggT_ps[:, bb * P:(bb + 1) * P],
                            lhsT=msgs[:, t, bb * C:(bb + 1) * C],
                            rhs=s_blk[:, t * P:(t + 1) * P],
                            start=(t == 0), stop=(t == T - 1),
                        )
                aggT_sb = agg_pool.tile([C, B * P], bf16)
                nc.vector.tensor_copy(out=aggT_sb[:], in_=aggT_ps[:])
                outT_ps = psO.tile([C, B * P], f32)
                for bb in range(B):
                    nc.tensor.matmul(
                        out=outT_ps[:, bb * P:(bb + 1) * P],
                        lhsT=wt_sb[:],
                        rhs=aggT_sb[:, bb * P:(bb + 1) * P],
                        start=True, stop=True)
                if blk % OUT_DMA_BLKS == 0:
                    ostage = ostage_pool.tile([C, B, OUT_DMA_BLKS * P], f32)
                o_off = (blk % OUT_DMA_BLKS) * P
                for bb in range(B):
                    nc.scalar.activation(
                        out=ostage[:, bb, o_off:o_off + P],
                        in_=outT_ps[:, bb * P:(bb + 1) * P],
                        func=mybir.ActivationFunctionType.Relu,
                        bias=bias_sb[:, :1], scale=1.0,
                    )
                if blk % OUT_DMA_BLKS == OUT_DMA_BLKS - 1 or blk == n_blocks - 1:
                    lo_blk = (blk // OUT_DMA_BLKS) * OUT_DMA_BLKS
                    width = (blk - lo_blk + 1) * P
                    for bb in range(B):
                        nc.sync.dma_start(
                            out=out_d[bb, :, lo_blk * P: lo_blk * P + width],
                            in_=ostage[:, bb, :width],
                        )
    return nc


def kernel(x, edge_row, edge_col, edge_vals, W, b):
    from concourse.bass_utils import run_bass_kernel_spmd

    x = np.asarray(x, np.float32)
    edge_row = np.asarray(edge_row, np.int32)
    edge_col = np.asarray(edge_col, np.int32)
    edge_vals = np.asarray(edge_vals, np.float32)
    W = np.asarray(W, np.float32)
    b = np.asarray(b, np.float32)

    lowidx, highidx, smat, counts, L, H, lmin, hmin = _preprocess(
        edge_row, edge_col, edge_vals)
    nc = _build_program(L, H, lmin, hmin)
    nc.compile()

    # xcat[n] = x[:, n, :] flattened -> [N, 4*128] bf16
    xcat = np.ascontiguousarray(
        x.transpose(1, 0, 2).reshape(N, B * C)).astype(ml_dtypes.bfloat16)
    wt = W.astype(ml_dtypes.bfloat16)
    in_maps = []
    for h in range(NCORES):
        in_maps.append({
            "xcat": xcat,
            "lowidx": np.ascontiguousarray(
                lowidx[h].transpose(1, 0, 2).reshape(P, RB * 8 * L)),
            "highidx": np.ascontiguousarray(
                highidx[h].transpose(1, 0, 2).reshape(P, RB * 8 * H)),
            "smat": smat[h],
            "counts": counts[h],
            "wt": wt,
            "bias": np.ascontiguousarray(b[:, None]),
        })

    res = run_bass_kernel_spmd(nc, in_maps, list(range(NCORES)))
    global LAST_RESULTS
    LAST_RESULTS = res

    out = np.empty((B, N, C), np.float32)
    for h in range(NCORES):
        lo, hi = h * RH, min((h + 1) * RH, N)
        o = res.results[h]["outT"]              # [B, C, RH]
        for bb in range(B):
            out[bb, lo:hi] = o[bb].T[:hi - lo]
    return out


# revision 25
# speedup vs baseline: 4.6474x; 1.1373x over previous
"""Trainium2 Bass kernel for GCNN message passing.

out[b] = relu((A @ x[b]) @ W + bias),  A sparse [N, N] from 800k edges.

Sharding (8 NeuronCores): core h owns output rows [h*6272, (h+1)*6272) for
ALL 4 batches. Host interleaves x into xcat[n] = x[:, n, :] (bf16,
[N, 4*128]) so ONE gather descriptor fetches a neighbor's features for all
4 batches at once (Q7 descriptor generation is the bottleneck resource, at
~8ns per gather index).

Device algorithm per core:
  Host pre-sorts the core's ~100k edges by destination row into 25
  row-blocks of 256 rows; within a block edges are split into "low"
  (col < 32768) / "high" groups so gather indices fit in int16, padded to
  uniform L / H tiles of 128 edges (col=0/val=0 padding).
  The scaled one-hot scatter matrices S[e, r] = (r == rl[e]) * val[e]
  ([128, 256] bf16 per edge-tile) are PREBUILT ON HOST and streamed in
  (DMA has headroom).
  Per row-block:
    - two dma_gather ops (bases xcat[0:], xcat[32768:]) fetch
      msgs [128(edge), T, 512] bf16; edge slot k -> partition k%128,
      tile k//128.
    - PE accumulates aggT_b[c, r] += msgs[:, t, b*128:+128].T @ S_t into
      PSUM [128, 4*256] f32 (segment sum via matmul accumulation).
    - aggT -> SBUF bf16, PE applies W (outT_b = W.T @ aggT_b) into a
      second PSUM tile, ACT applies relu(.+bias), batched DMA writes
      outT [4, 128, 6400] f32.
  Host transposes/concatenates the 8 per-core outputs.
"""
import sys

import numpy as np

try:  # concourse (Bass) lives in the trn repo
    import concourse  # noqa: F401
except ImportError:  # pragma: no cover
    sys.path.insert(0, "/opt/trn_rl_repo")

import ml_dtypes

B, N, E, C = 4, 50000, 800000, 128
LAST_RESULTS = None  # BassKernelResults of the most recent kernel() call
P = 128
BR = 256            # rows per block
RB = 25             # row-blocks per core (covers 6400 >= 6272 rows)
RH = 6272           # row stride between cores (8 * 6272 = 50176 >= N)
NCORES = 8
SPLIT = 32768       # low/high column split for int16 gather indices
OUT_DMA_BLKS = 4    # row-blocks per output DMA


def _pack_idx(vals, n_slots):
    """dma_gather int16 index layout: index k at [k % 16, k // 16],
    replicated to 128 partitions; 0-padded. -> [128, n_slots // 16]"""
    buf = np.zeros(n_slots, np.int16)
    buf[:len(vals)] = vals
    tile16 = buf.reshape(n_slots // 16, 16).T
    return np.tile(tile16, (8, 1))


def _preprocess(edge_row, edge_col, edge_vals):
    """Per-core gather-index tables and host-built S matrices.

    Returns (lowidx [8, RB, 128, 8L], highidx [8, RB, 128, 8H],
             smat [8, 128, RB*T*BR] bf16, L, H).
    Edge slot k of a block: partition k%128, tile k//128; slots < L*128
    low-group (col), the rest high-group (col - SPLIT).
    S tile t of block blk lives at smat[:, (blk*T+t)*BR:(blk*T+t+1)*BR].
    """
    per_core = []
    maxlow = maxhigh = 0
    for h in range(NCORES):
        lo, hi = h * RH, min((h + 1) * RH, N)
        m = (edge_row >= lo) & (edge_row < hi)
        r, c, v = edge_row[m] - lo, edge_col[m], edge_vals[m]
        is_high = c >= SPLIT
        order = np.lexsort((is_high, r // BR))
        r, c, v, is_high = r[order], c[order], v[order], is_high[order]
        blocks = []
        for blk in range(RB):
            sel = slice(*np.searchsorted(r // BR, [blk, blk + 1]))
            rb, cb, vb, hb = r[sel], c[sel], v[sel], is_high[sel]
            nlow = int((~hb).sum())
            blocks.append((rb, cb, vb, nlow))
            maxlow = max(maxlow, nlow)
            maxhigh = max(maxhigh, len(rb) - nlow)
        per_core.append(blocks)
    L = (maxlow + P - 1) // P
    H = (maxhigh + P - 1) // P
    T = L + H
    lowidx = np.zeros((NCORES, RB, P, 8 * L), np.int16)
    highidx = np.zeros((NCORES, RB, P, 8 * H), np.int16)
    smat = np.zeros((NCORES, P, RB * T * BR), ml_dtypes.bfloat16)
    iota = np.arange(BR, dtype=np.float32)
    for h in range(NCORES):
        for blk in range(RB):
            rb, cb, vb, nlow = per_core[h][blk]
            nh = len(rb) - nlow
            lowidx[h, blk] = _pack_idx(cb[:nlow], L * P)
            highidx[h, blk] = _pack_idx(cb[nlow:] - SPLIT, H * P)
            rr = np.zeros(T * P, np.float32)
            vv = np.zeros(T * P, np.float32)
            rr[:nlow] = (rb[:nlow] - blk * BR).astype(np.float32)
            vv[:nlow] = vb[:nlow]
            rr[L * P:L * P + nh] = (rb[nlow:] - blk * BR).astype(np.float32)
            vv[L * P:L * P + nh] = vb[nlow:]
            # S[e, r] for slot e=t*P+p -> smat[p, (blk*T+t)*BR + r]
            s_f32 = (iota[None, :] == rr[:, None]) * vv[:, None]  # [T*P, BR]
            smat[h, :, blk * T * BR:(blk + 1) * T * BR] = (
                s_f32.reshape(T, P, BR).transpose(1, 0, 2).reshape(P, T * BR)
                .astype(ml_dtypes.bfloat16))
    return lowidx, highidx, smat, L, H


def _build_program(L, H, n_blocks=RB, n_rows=N):
    import concourse.bacc as bacc
    import concourse.tile as tile
    from concourse import mybir
    from concourse._compat import get_trn_type

    T = L + H
    BC = B * C                       # 512 feature cols in xcat
    f32 = mybir.dt.float32
    bf16 = mybir.dt.bfloat16
    i16 = mybir.dt.int16
    nc = bacc.Bacc(get_trn_type() or "TRN2", target_bir_lowering=False)

    x_d = nc.dram_tensor("xcat", [n_rows, BC], bf16, kind="ExternalInput")
    lowidx_d = nc.dram_tensor("lowidx", [P, n_blocks * 8 * L], i16,
                              kind="ExternalInput")
    highidx_d = nc.dram_tensor("highidx", [P, n_blocks * 8 * H], i16,
                               kind="ExternalInput")
    smat_d = nc.dram_tensor("smat", [P, n_blocks * T * BR], bf16,
                            kind="ExternalInput")
    wt_d = nc.dram_tensor("wt", [C, C], bf16, kind="ExternalInput")
    bias_d = nc.dram_tensor("bias", [C, 1], f32, kind="ExternalInput")
    out_d = nc.dram_tensor("outT", [B, C, n_blocks * BR], f32,
                           kind="ExternalOutput")

    with tile.TileContext(nc) as tc:
        with (
            tc.tile_pool(name="const", bufs=1) as const_pool,
            tc.tile_pool(name="meta", bufs=1) as meta_pool,
            tc.tile_pool(name="msgs", bufs=3) as msgs_pool,
            tc.tile_pool(name="smat", bufs=3) as s_pool,
            tc.tile_pool(name="aggsb", bufs=2) as agg_pool,
            tc.tile_pool(name="ostage", bufs=2) as ostage_pool,
            tc.tile_pool(name="psum_agg", bufs=2, space="PSUM") as psA,
            tc.tile_pool(name="psum_out", bufs=2, space="PSUM") as psO,
        ):
            wt_sb = const_pool.tile([C, C], bf16)
            bias_sb = const_pool.tile([C, 1], f32)
            nc.sync.dma_start(out=wt_sb[:], in_=wt_d[:])
            nc.sync.dma_start(out=bias_sb[:], in_=bias_d[:])

            lowidx_sb = meta_pool.tile([P, n_blocks * 8 * L], i16)
            highidx_sb = meta_pool.tile([P, n_blocks * 8 * H], i16)
            nc.sync.dma_start(out=lowidx_sb[:], in_=lowidx_d[:])
            nc.sync.dma_start(out=highidx_sb[:], in_=highidx_d[:])

            ostage = None
            for blk in range(n_blocks):
                msgs = msgs_pool.tile([P, T, BC], bf16)
                nc.gpsimd.dma_gather(
                    out_ap=msgs[:, :L, :],
                    in_ap=x_d[:SPLIT, :],
                    idxs_ap=lowidx_sb[:, blk * 8 * L:(blk + 1) * 8 * L],
                    num_idxs=L * P,
                    num_idxs_reg=L * P,
                    elem_size=BC,
                    single_packet=False,
                )
                nc.gpsimd.dma_gather(
                    out_ap=msgs[:, L:, :],
                    in_ap=x_d[SPLIT:, :],
                    idxs_ap=highidx_sb[:, blk * 8 * H:(blk + 1) * 8 * H],
                    num_idxs=H * P,
                    num_idxs_reg=H * P,
                    elem_size=BC,
                    single_packet=False,
                )
                s_blk = s_pool.tile([P, T * BR], bf16)
                nc.sync.dma_start(
                    out=s_blk[:],
                    in_=smat_d[:, blk * T * BR:(blk + 1) * T * BR])
                aggT_ps = psA.tile([C, B * BR], f32)
                for bb in range(B):
                    for t in range(T):
                        nc.tensor.matmul(
                            out=aggT_ps[:, bb * BR:(bb + 1) * BR],
                            lhsT=msgs[:, t, bb * C:(bb + 1) * C],
                            rhs=s_blk[:, t * BR:(t + 1) * BR],
                            start=(t == 0), stop=(t == T - 1),
                        )
                aggT_sb = agg_pool.tile([C, B * BR], bf16)
                nc.vector.tensor_copy(out=aggT_sb[:], in_=aggT_ps[:])
                outT_ps = psO.tile([C, B * BR], f32)
                for bb in range(B):
                    nc.tensor.matmul(
                        out=outT_ps[:, bb * BR:(bb + 1) * BR],
                        lhsT=wt_sb[:],
                        rhs=aggT_sb[:, bb * BR:(bb + 1) * BR],
                        start=True, stop=True)
                if blk % OUT_DMA_BLKS == 0:
                    ostage = ostage_pool.tile([C, B, OUT_DMA_BLKS * BR], f32)
                o_off = (blk % OUT_DMA_BLKS) * BR
                for bb in range(B):
                    nc.scalar.activation(
                        out=ostage[:, bb, o_off:o_off + BR],
                        in_=outT_ps[:, bb * BR:(bb + 1) * BR],
                        func=mybir.ActivationFunctionType.Relu,
                        bias=bias_sb[:, :1], scale=1.0,
                    )
                if blk % OUT_DMA_BLKS == OUT_DMA_BLKS - 1 or blk == n_blocks - 1:
                    lo_blk = (blk // OUT_DMA_BLKS) * OUT_DMA_BLKS
                    width = (blk - lo_blk + 1) * BR
                    for bb in range(B):
                        nc.sync.dma_start(
                            out=out_d[bb, :, lo_blk * BR: lo_blk * BR + width],
                            in_=ostage[:, bb, :width],
                        )
    return nc


def kernel(x, edge_row, edge_col, edge_vals, W, b):
    from concourse.bass_utils import run_bass_kernel_spmd

    x = np.asarray(x, np.float32)
    edge_row = np.asarray(edge_row, np.int32)
    edge_col = np.asarray(edge_col, np.int32)
    edge_vals = np.asarray(edge_vals, np.float32)
    W = np.asarray(W, np.float32)
    b = np.asarray(b, np.float32)

    lowidx, highidx, smat, L, H = _preprocess(edge_row, edge_col, edge_vals)
    nc = _build_program(L, H)
    nc.compile()

    # xcat[n] = x[:, n, :] flattened -> [N, 4*128] bf16
    xcat = np.ascontiguousarray(
        x.transpose(1, 0, 2).reshape(N, B * C)).astype(ml_dtypes.bfloat16)
    wt = W.astype(ml_dtypes.bfloat16)
    in_maps = []
    for h in range(NCORES):
        in_maps.append({
            "xcat": xcat,
            "lowidx": np.ascontiguousarray(
                lowidx[h].transpose(1, 0, 2).reshape(P, RB * 8 * L)),
            "highidx": np.ascontiguousarray(
                highidx[h].transpose(1, 0, 2).reshape(P, RB * 8 * H)),
            "smat": smat[h],
            "wt": wt,
            "bias": np.ascontiguousarray(b[:, None]),
        })

    res = run_bass_kernel_spmd(nc, in_maps, list(range(NCORES)))
    global LAST_RESULTS
    LAST_RESULTS = res

    out = np.empty((B, N, C), np.float32)
    for h in range(NCORES):
        lo, hi = h * RH, min((h + 1) * RH, N)
        o = res.results[h]["outT"]              # [B, C, RB*BR]
        for bb in range(B):
            out[bb, lo:hi] = o[bb].T[:hi - lo]
    return out
